# revision 36
# baseline (speedup 1.0000x reference)
"""2-layer GCN (GCNConv 128->128->64, N=50000, E=800000) on 8 TRN2 NeuronCores.

v6 strategy (dst-sharded, aggregate-first, unified bf16 chunk tables):
  out = relu(A_hat @ (relu(A_hat @ x @ W1 + b1)) @ W2 + b2),  A_hat = D^-1/2 (A+I) D^-1/2
  - BOTH layers gather pre-scaled bf16 tables laid out identically:
      layer 1: x1' = Dis*x      (built LOCALLY per core from the replicated
               x input — dense read/scale/write, no collective, no barrier)
      layer 2: x2' = Dis*relu(h1)  (2 chunked AllGathers mid-L1, ~35us each)
    so the two layers share ONE host-side edge plan (same idx/dr arrays,
    same one-hot structure), need no per-edge scale pass (dis[src] lives in
    the table, dis[dst] is applied post-aggregation via a replicated row
    table), and skip the old f32->bf16 ScalarE conversion entirely.
  - One-hots [edge, dst, tile] built in a single is_equal pass (2x DVE mode).
  - Edges sorted by dst, sharded across 8 cores by dst range (6250/core),
    dst blocks of 128, groups of GS=4 blocks; scatter-add via
    PSUM-accumulated TensorE matmuls; self loops via stage @ identity in
    pass 1; gathers in 8-tile single-packet dma_gather calls round-robined
    over 4 SWDGE queues (measured fastest vs all bigger-call variants).
  - Both layers run two passes (one per chunk table) accumulating into a
    bf16 SBUF accumulator; layer-2 chunk-0 AllGather is emitted mid-layer-1
    pass 1, chunk-1 after layer-2 pass 0's gather desc-gen.
Host-side work is index-only prep (sort/degree-histogram/plan) + output concat.
"""

import numpy as np
import ml_dtypes

import concourse.bass as bass
import concourse.bacc as bacc
import concourse.mybir as mybir
import concourse.tile as tile
from concourse.bass_utils import run_bass_kernel_spmd
from concourse.library_config import mlp
from concourse.masks import make_identity

P = 128
N_NODES = 50000
N_EDGES = 800000
IN_CH = 128
HID_CH = 128
OUT_CH = 64
N_CORES = 8
NSH = N_NODES // N_CORES          # 6250 nodes per core
NBLK = (NSH + P - 1) // P         # 49 dst blocks per core (48 full + 106)
NFULL = NSH // P                  # 48
NTAIL = NSH - NFULL * P           # 106
GS = 4                            # dst blocks per group
NG = (NBLK + GS - 1) // GS        # 13 groups (last has 1 block)
GCAP = 8                          # max tiles per dma_gather call
CLEN = [25, 24]                   # src-block chunks (blocks 0:25, 25:49)
CSTART = [0, 25]
AG_SPLIT = 7                      # x2 chunk-0 AllGather after this many
                                  # layer-1 pass-1 groups (covers blocks 0:28)

BF16 = mybir.dt.bfloat16
F32 = mybir.dt.float32

LAST_RESULT = None  # for test harness: BassKernelResults of last run
REPEAT = 1          # measurement aid: unroll the kernel body this many times


def _plan(cnt):
    t = np.ceil(cnt.max(axis=0) / P).astype(np.int64)
    off = np.concatenate([[0], np.cumsum(t)])[:-1]
    return t, off, int(t.sum())


def _host_prep(edge_index):
    """Index-only preprocessing. Returns per-core upload arrays + tile plan."""
    src = edge_index[0].astype(np.int64)
    dst = edge_index[1].astype(np.int64)

    deg = np.bincount(dst, minlength=N_NODES) + 1
    dis = (1.0 / np.sqrt(deg.astype(np.float64))).astype(np.float64)

    core = dst // NSH
    i_dst = dst - core * NSH
    blk = i_dst // P
    grp = blk // GS
    big = blk - grp * GS

    # full-graph dis in block-column layout: discol[p, k*NBLK+b] = dis[node]
    discol = np.zeros((P, N_CORES * NBLK), np.float32)
    nodes = np.arange(N_NODES)
    kk_ = nodes // NSH
    ii_ = nodes - kk_ * NSH
    discol[ii_ % P, kk_ * NBLK + ii_ // P] = dis[nodes]

    # chunked tables; row = (ksrc*128+psrc)*len_c + (bsrc-cstart)
    ksrc = src // NSH
    isrc = src - ksrc * NSH
    bsrc = isrc // P
    psrc = isrc - bsrc * P
    ch = (bsrc >= CLEN[0]).astype(np.int64)
    lenc = np.where(ch == 1, CLEN[1], CLEN[0])
    rows = (ksrc * P + psrc) * lenc + (bsrc - ch * CSTART[1])
    kk = ((core * NG + grp) * 2 + ch) * GS + big
    NSLOT = NG * 2 * GS                                   # 104 per core
    cnt = np.zeros((N_CORES, NSLOT), np.int64)
    np.add.at(cnt, (core, kk - core * NSLOT), 1)
    t, off, T = _plan(cnt)

    drel = (dst - (core * NSH + blk * P)).astype(np.float32)

    # secondary sort by table row: each gather call's descriptors become
    # ascending in address (better DRAM behavior; order within a section is
    # free since scatter-add commutes)
    order = np.lexsort((rows, kk))
    ks = kk[order]
    group_start = np.concatenate([[0], np.cumsum(cnt.reshape(-1))])[:-1]
    pos = np.arange(len(ks)) - group_start[ks]
    slot_in_core = off[ks % NSLOT] * P + pos
    ecore = ks // NSLOT
    EPC = T * P
    idx_rows = np.zeros((N_CORES, EPC), np.int64)
    dr = np.full((N_CORES, EPC), -1.0, np.float32)
    idx_rows[ecore, slot_in_core] = rows[order]
    dr[ecore, slot_in_core] = drel[order]
    idxw = np.zeros((N_CORES, 16, T * 8), np.int16)
    ii = np.arange(EPC)
    for k in range(N_CORES):
        w = np.zeros((16, T * 8), np.int16)
        w[ii % 16, ii // 16] = idx_rows[k]
        idxw[k] = w
    idxw = np.tile(idxw, (1, 8, 1))
    drw = dr.reshape(N_CORES, T, P).transpose(0, 2, 1)

    disw = np.zeros((N_CORES, P, NBLK), np.float32)
    diswr = np.zeros((N_CORES, NBLK * P), np.float32)
    nodes = np.arange(NBLK * P)
    valid = nodes < NSH
    disf = dis.astype(np.float32)
    for k in range(N_CORES):
        v = np.zeros(NBLK * P, np.float32)
        v[valid] = disf[k * NSH + nodes[valid]]
        disw[k] = v.reshape(NBLK, P).T
        diswr[k] = v

    return {
        "t": t, "off": off, "T": T,
        "idxw": idxw.astype(np.int16),
        "dr": drw.astype(ml_dtypes.bfloat16),
        "disw": disw, "diswr": diswr.astype(ml_dtypes.bfloat16),
        "discol": discol,
    }


def _make_in_maps(x, W1, b1, W2, b2, prep):
    in_maps = []
    xf = np.ascontiguousarray(x, dtype=np.float32)
    for k in range(N_CORES):
        in_maps.append({
            "x": xf,
            "xsh": np.ascontiguousarray(xf[k * NSH:(k + 1) * NSH]),
            "discol": np.ascontiguousarray(prep["discol"]),
            "w1": np.ascontiguousarray(W1, dtype=np.float32),
            "b1": np.ascontiguousarray(np.asarray(b1, np.float32).reshape(HID_CH, 1)),
            "b1row": np.ascontiguousarray(
                np.tile(np.asarray(b1, np.float32)[None, :], (P, 1))),
            "b2row": np.ascontiguousarray(
                np.tile(np.asarray(b2, np.float32)[None, :], (P, 1))),
            "w2": np.ascontiguousarray(W2, dtype=np.float32),
            "b2": np.ascontiguousarray(np.asarray(b2, np.float32).reshape(OUT_CH, 1)),
            "disw": np.ascontiguousarray(prep["disw"][k]),
            "idxw": np.ascontiguousarray(prep["idxw"][k]),
            "dr": np.ascontiguousarray(prep["dr"][k]),
            "diswr": np.ascontiguousarray(
                np.tile(prep["diswr"][k][None, :], (P, 1))),
        })
    return in_maps


def _build(prep):
    t, off, T = prep["t"], prep["off"], prep["T"]

    nc = bacc.Bacc("TRN2", target_bir_lowering=False, num_devices=N_CORES,
                   num_swdge_queues=4)

    t_x = nc.dram_tensor("x", [N_NODES, IN_CH], F32, kind="ExternalInput")
    t_xsh = nc.dram_tensor("xsh", [NSH, IN_CH], F32, kind="ExternalInput")
    t_discol = nc.dram_tensor("discol", [P, N_CORES * NBLK], F32,
                              kind="ExternalInput")
    t_w1 = nc.dram_tensor("w1", [IN_CH, HID_CH], F32, kind="ExternalInput")
    t_b1row = nc.dram_tensor("b1row", [P, HID_CH], F32, kind="ExternalInput")
    t_w2 = nc.dram_tensor("w2", [HID_CH, OUT_CH], F32, kind="ExternalInput")
    t_b2row = nc.dram_tensor("b2row", [P, OUT_CH], F32, kind="ExternalInput")
    t_disw = nc.dram_tensor("disw", [P, NBLK], F32, kind="ExternalInput")
    t_idx = nc.dram_tensor("idxw", [P, T * 8], mybir.dt.int16, kind="ExternalInput")
    t_dr = nc.dram_tensor("dr", [P, T], BF16, kind="ExternalInput")
    t_diswr = nc.dram_tensor("diswr", [P, NBLK * P], BF16, kind="ExternalInput")
    t_out = nc.dram_tensor("out", [NSH, OUT_CH], F32, kind="ExternalOutput")

    x1f = [nc.dram_tensor(f"x1f_{c}", [N_CORES * P, CLEN[c] * IN_CH], BF16)
           for c in range(2)]
    x2s = [nc.dram_tensor(f"x2s_{c}", [P, CLEN[c] * HID_CH], BF16) for c in range(2)]
    # Shared addr_space enables the 1-hop shared-output AllGather path
    x2f = [nc.dram_tensor(f"x2f_{c}", [N_CORES * P, CLEN[c] * HID_CH], BF16,
                          addr_space="Shared")
           for c in range(2)]

    rg = [list(range(N_CORES))]

    def slot(g, c, big):
        return (g * 2 + c) * GS + big

    meta = []
    for g in range(NG):
        bigs = list(range(min(GS, NBLK - g * GS)))
        Tg = int(off[slot(g, 0, 0)])
        S = [0, 0]
        rel = {}
        pos = 0
        for c in range(2):
            for big in bigs:
                n = int(t[slot(g, c, big)])
                rel[(c, big)] = (pos, n)
                S[c] += n
                pos += n
        meta.append((Tg, S, rel, bigs))
    SMAX = max(max(S) for _, S, _, _ in meta)

    with tile.TileContext(nc) as tc:
        with (
            tc.tile_pool(name="const", bufs=1) as cp,
            tc.tile_pool(name="stage", bufs=2) as stp,
            tc.tile_pool(name="sbuf", bufs=4) as sb,
            tc.tile_pool(name="gb", bufs=4) as gb,      # gathered bf16 tiles
            tc.tile_pool(name="ob", bufs=3) as ob,      # one-hot blocks
            tc.tile_pool(name="psum", bufs=3, space="PSUM") as ps,
            tc.tile_pool(name="psA", bufs=4, space="PSUM") as psA,
        ):
            nc.gpsimd.load_library(mlp)

            idx_sb = cp.tile([P, T * 8], mybir.dt.int16)
            nc.sync.dma_start(out=idx_sb[:], in_=t_idx[:, :])
            discol_sb = cp.tile([P, N_CORES * NBLK], F32)
            nc.sync.dma_start(out=discol_sb[:], in_=t_discol[:, :])
            # dr as [P, 1, T] so middle-dim broadcast slices need no None-axis
            dr_sb = cp.tile([P, 1, T], BF16)
            nc.sync.dma_start(out=dr_sb[:, 0, :], in_=t_dr[:, :])
            # dis[dst] in row layout (replicated over partitions): applied
            # post-aggregation per block
            diswr_sb = cp.tile([P, NBLK * P], BF16)
            nc.sync.dma_start(out=diswr_sb[:], in_=t_diswr[:, :])
            disw_sb = cp.tile([P, NBLK], F32)
            nc.sync.dma_start(out=disw_sb[:, :], in_=t_disw[:, :])

            iota_i = cp.tile([P, P], mybir.dt.int32)
            nc.gpsimd.iota(iota_i[:], pattern=[[1, P]], base=0, channel_multiplier=0)
            iota_bf = cp.tile([P, 1, P], BF16)
            nc.vector.tensor_copy(out=iota_bf[:, 0, :], in_=iota_i[:])
            # materialized iota over the dst (middle) axis: val[p, d, j] = d
            iota_big = cp.tile([P, P, SMAX], BF16)
            nc.vector.tensor_copy(
                out=iota_big[:],
                in_=iota_bf[:, 0, :][:, :, None].to_broadcast([P, P, SMAX]))

            ident_bf = cp.tile([P, P], BF16)
            make_identity(nc, ident_bf[:])

            w1_f = cp.tile([IN_CH, HID_CH], F32)
            nc.sync.dma_start(out=w1_f[:], in_=t_w1[:, :])
            w1_bf = cp.tile([IN_CH, HID_CH], BF16)
            nc.vector.tensor_copy(out=w1_bf[:], in_=w1_f[:])
            w2_f = cp.tile([HID_CH, OUT_CH], F32)
            nc.sync.dma_start(out=w2_f[:], in_=t_w2[:, :])
            w2_bf = cp.tile([HID_CH, OUT_CH], BF16)
            nc.vector.tensor_copy(out=w2_bf[:], in_=w2_f[:])
            b1row_sb = cp.tile([P, HID_CH], F32)
            nc.sync.dma_start(out=b1row_sb[:], in_=t_b1row[:, :])
            b2row_sb = cp.tile([P, OUT_CH], F32)
            nc.sync.dma_start(out=b2row_sb[:], in_=t_b2row[:, :])

            x1stage = cp.tile([P, NBLK, IN_CH], BF16)
            x2st0 = cp.tile([P, CLEN[0], HID_CH], BF16)
            x2st1 = cp.tile([P, CLEN[1], HID_CH], BF16)
            x2st = [x2st0, x2st1]

            def x2stage_slice(b):
                c = 0 if b < CSTART[1] else 1
                return x2st[c][:, b - CSTART[c], :]

            acc1 = cp.tile([HID_CH, NBLK, P], BF16)
            acc2 = cp.tile([HID_CH, NBLK, P], BF16)
            acc = [acc1, acc2]

            gq = [0]

            def gather(out_ap, in_ap, t0, n):
                o0 = 0
                while o0 < n:
                    m = min(GCAP, n - o0)
                    nc.gpsimd.dma_gather(
                        out_ap=out_ap[:, o0:o0 + m, :], in_ap=in_ap,
                        idxs_ap=idx_sb[:, 8 * (t0 + o0): 8 * (t0 + o0 + m)],
                        num_idxs=m * P, num_idxs_reg=m * P, elem_size=IN_CH,
                        queue_num=gq[0] % 4, single_packet=True,
                    )
                    gq[0] += 1
                    o0 += m

            def onehot(S, Tg):
                """[P(edge), P(dst), S(tile)] one-hot."""
                o3 = ob.tile([P, P, S], BF16, tag="o3")
                nc.vector.tensor_tensor(
                    out=o3[:], in0=iota_big[:, :, :S],
                    in1=dr_sb[:, :, Tg:Tg + S].to_broadcast([P, P, S]),
                    op=mybir.AluOpType.is_equal,
                )
                return o3

            def epi1(b, ups):
                # ups: [P dst, HID] psum. x2' = Dis*relu(t+b1), relu/scale
                # commute (dis>0) so ACT applies the per-partition dis scale.
                tmp2 = sb.tile([P, HID_CH], F32, tag="ep1")
                nc.vector.tensor_tensor(out=tmp2[:], in0=ups[:],
                                        in1=b1row_sb[:],
                                        op=mybir.AluOpType.add)
                nc.scalar.activation(out=x2stage_slice(b), in_=tmp2[:],
                                     func=mybir.ActivationFunctionType.Relu,
                                     scale=disw_sb[:, b: b + 1])

            def epi2(b, ups):
                tmp2 = sb.tile([P, OUT_CH], F32, tag="ep2")
                nc.vector.tensor_tensor(out=tmp2[:], in0=ups[:],
                                        in1=b2row_sb[:],
                                        op=mybir.AluOpType.add)
                outt = sb.tile([P, OUT_CH], F32, tag="outt")
                nc.scalar.activation(out=outt[:], in_=tmp2[:],
                                     func=mybir.ActivationFunctionType.Relu)
                nb = P if b < NFULL else NTAIL
                nc.sync.dma_start(out=t_out[b * P: b * P + nb, :], in_=outt[:nb, :])

            def do_group(l, tabs, stage_slice, wl, epi, g, cpass):
                Tg, S, rel, bigs = meta[g]
                Sc = S[cpass]
                base = Tg + (S[0] if cpass == 1 else 0)
                if Sc:
                    g2 = gb.tile([P, Sc, HID_CH], BF16, tag="gb")
                    gather(g2[:, :, :], tabs[cpass], base, Sc)
                    o3 = onehot(Sc, base)
                for big in bigs:
                    b = g * GS + big
                    p0, n = rel[(cpass, big)]
                    p0 -= S[0] if cpass == 1 else 0
                    if cpass == 0:
                        if n == 0:
                            nc.vector.memset(acc[l][:, b, :], 0.0)
                            continue
                        tps = psA.tile([HID_CH, P], F32, tag="tps")
                        for ji in range(n):
                            nc.tensor.matmul(out=tps[:], lhsT=g2[:, p0 + ji, :],
                                             rhs=o3[:, :, p0 + ji],
                                             start=(ji == 0), stop=(ji == n - 1))
                        nc.vector.tensor_tensor(
                            out=acc[l][:, b, :], in0=tps[:],
                            in1=diswr_sb[:, b * P:(b + 1) * P],
                            op=mybir.AluOpType.mult)
                    else:
                        tps = psA.tile([HID_CH, P], F32, tag="tps")
                        for ji in range(n):
                            nc.tensor.matmul(out=tps[:], lhsT=g2[:, p0 + ji, :],
                                             rhs=o3[:, :, p0 + ji],
                                             start=(ji == 0), stop=False)
                        nc.tensor.matmul(out=tps[:], lhsT=stage_slice(b),
                                         rhs=ident_bf[:], start=(n == 0),
                                         stop=True)
                        tmp = sb.tile([HID_CH, P], F32, tag="tmp")
                        nc.vector.tensor_tensor(
                            out=tmp[:], in0=tps[:],
                            in1=diswr_sb[:, b * P:(b + 1) * P],
                            op=mybir.AluOpType.mult)
                        t_sb = sb.tile([HID_CH, P], BF16, tag="tsb")
                        nc.vector.tensor_tensor(
                            out=t_sb[:], in0=tmp[:], in1=acc[l][:, b, :],
                            op=mybir.AluOpType.add)
                        ups = ps.tile([P, wl[1]], F32, tag="ups")
                        nc.tensor.matmul(out=ups[:], lhsT=t_sb[:], rhs=wl[0][:],
                                         start=True, stop=True)
                        epi(b, ups)

            tab1 = [x1f[c].ap().rearrange("q (r c) -> (q r) c", c=IN_CH)
                    for c in range(2)]
            tab2 = [x2f[c].ap().rearrange("q (r c) -> (q r) c", c=HID_CH)
                    for c in range(2)]
            x1stage_slice = lambda b: x1stage[:, b, :]

            x1fv = [x1f[c].ap().rearrange("q (b c) -> q b c", c=IN_CH)
                    for c in range(2)]
            RUNS = [[(0, 13), (13, 12)],
                    [(25, 12), (37, 12)]]
            QS = [(0, 13), (13, 12), (25, 12), (37, 12)]
            for _rep in range(REPEAT):
                # layer-1 table x1' = Dis*x, built LOCALLY on every core from
                # the replicated x input (no collective): dense chunked
                # read -> scale -> bf16 -> strided write into table layout
                for c in range(2):
                    for k in range(N_CORES):
                        for b0, run in RUNS[c]:
                            base = k * NSH + b0 * P
                            bx = stp.tile([P, 13, IN_CH], F32, tag="bx1")
                            if b0 + run == NBLK:
                                nc.vector.memset(bx[:, run - 1, :], 0.0)
                                nc.sync.dma_start(
                                    out=bx[:, :run - 1, :],
                                    in_=t_x[base: base + (run - 1) * P, :]
                                    .rearrange("(b p) c -> p b c", p=P))
                                nc.sync.dma_start(
                                    out=bx[:NTAIL, run - 1, :],
                                    in_=t_x[base + (run - 1) * P:
                                            (k + 1) * NSH, :])
                            else:
                                nc.sync.dma_start(
                                    out=bx[:, :run, :],
                                    in_=t_x[base: base + run * P, :]
                                    .rearrange("(b p) c -> p b c", p=P))
                            bxh = stp.tile([P, 13, IN_CH], BF16, tag="bx1h")
                            nc.vector.tensor_tensor(
                                out=bxh[:, :run, :], in0=bx[:, :run, :],
                                in1=discol_sb[:, k * NBLK + b0:
                                              k * NBLK + b0 + run][:, :, None]
                                .to_broadcast([P, run, IN_CH]),
                                op=mybir.AluOpType.mult)
                            nc.sync.dma_start(
                                out=x1fv[c][k * P:(k + 1) * P,
                                            b0 - CSTART[c]:
                                            b0 - CSTART[c] + run, :],
                                in_=bxh[:, :run, :])

                # own-shard stage (self loops): Dis*x in [node, block, ch]
                for q0, qn in QS:
                    sbx = stp.tile([P, 13, IN_CH], F32, tag="sbx")
                    if q0 + qn == NBLK:
                        nc.vector.memset(sbx[:, qn - 1, :], 0.0)
                        nc.sync.dma_start(
                            out=sbx[:, :qn - 1, :],
                            in_=t_xsh[q0 * P: (q0 + qn - 1) * P, :]
                            .rearrange("(b p) c -> p b c", p=P))
                        nc.sync.dma_start(out=sbx[:NTAIL, qn - 1, :],
                                          in_=t_xsh[(q0 + qn - 1) * P:, :])
                    else:
                        nc.sync.dma_start(
                            out=sbx[:, :qn, :],
                            in_=t_xsh[q0 * P: (q0 + qn) * P, :]
                            .rearrange("(b p) c -> p b c", p=P))
                    nc.vector.tensor_tensor(
                        out=x1stage[:, q0:q0 + qn, :], in0=sbx[:, :qn, :],
                        in1=disw_sb[:, q0:q0 + qn][:, :, None]
                        .to_broadcast([P, qn, IN_CH]),
                        op=mybir.AluOpType.mult)

                # ================= Layer 1 =================
                for g in range(NG):
                    do_group(0, tab1, x1stage_slice, (w1_bf, HID_CH), epi1, g, 0)
                for g in range(AG_SPLIT):
                    do_group(0, tab1, x1stage_slice, (w1_bf, HID_CH), epi1, g, 1)
                # chunk-0 x2' ready (blocks 0:28 done): exchange while the
                # rest of layer-1 pass 1 runs
                nc.sync.dma_start(
                    out=x2s[0][:, :], in_=x2st[0][:].rearrange("p b c -> p (b c)"))
                nc.gpsimd.collective_compute(
                    "AllGather", mybir.AluOpType.bypass, replica_groups=rg,
                    ins=[x2s[0].ap().opt()], outs=[x2f[0].ap().opt()],
                )
                for g in range(AG_SPLIT, NG):
                    do_group(0, tab1, x1stage_slice, (w1_bf, HID_CH), epi1, g, 1)
                nc.sync.dma_start(
                    out=x2s[1][:, :], in_=x2st[1][:].rearrange("p b c -> p (b c)"))

                # ================= Layer 2 =================
                for g in range(NG):
                    do_group(1, tab2, x2stage_slice, (w2_bf, OUT_CH), epi2, g, 0)
                # chunk-1 AllGather after pass-0's gather desc-gen so those
                # gathers aren't queued behind it on the Pool engine
                nc.gpsimd.collective_compute(
                    "AllGather", mybir.AluOpType.bypass, replica_groups=rg,
                    ins=[x2s[1].ap().opt()], outs=[x2f[1].ap().opt()],
                )
                for g in range(NG):
                    do_group(1, tab2, x2stage_slice, (w2_bf, OUT_CH), epi2, g, 1)

    nc.compile()
    return nc


_BUILD_CACHE = {}


def kernel(x, edge_index, W1, b1, W2, b2, _trace=False):
    global LAST_RESULT
    x = np.asarray(x, dtype=np.float32)
    edge_index = np.asarray(edge_index, dtype=np.int32)

    # memoize prep+program per edge structure: repeated harness calls skip
    # host planning and Bass program construction (the NEFF itself is also
    # cached downstream by the compiler cache)
    ekey = hash(edge_index.tobytes())
    if ekey not in _BUILD_CACHE:
        prep = _host_prep(edge_index)
        _BUILD_CACHE[ekey] = (prep, _build(prep))
    prep, nc = _BUILD_CACHE[ekey]
    in_maps = _make_in_maps(x, W1, b1, W2, b2, prep)

    res = None
    for attempt in range(3):
        try:
            res = run_bass_kernel_spmd(nc, in_maps, core_ids=list(range(N_CORES)),
                                       trace=_trace)
            break
        except Exception:
            # transient axon/device hiccups (e.g. NRT_EXEC_UNIT_UNRECOVERABLE
            # left by a prior run) usually clear on re-execution — but the
            # poisoned PJRT client must be dropped first
            if attempt == 2:
                raise
            import time as _time
            try:
                import jax as _jax
                _jax.clear_caches()
                _jax.extend.backend.clear_backends()
            except Exception:
                pass
            _time.sleep(10)
    LAST_RESULT = res
    out = np.concatenate([res.results[k]["out"] for k in range(N_CORES)], axis=0)
    return out.astype(np.float32)


# revision 37
# speedup vs baseline: 1.0663x; 1.0663x over previous
"""2-layer GCN (GCNConv 128->128->64, N=50000, E=800000) on 8 TRN2 NeuronCores.

v6 strategy (dst-sharded, aggregate-first, unified bf16 chunk tables):
  out = relu(A_hat @ (relu(A_hat @ x @ W1 + b1)) @ W2 + b2),  A_hat = D^-1/2 (A+I) D^-1/2
  - BOTH layers gather pre-scaled bf16 tables laid out identically:
      layer 1: x1' = Dis*x      (built LOCALLY per core from the replicated
               x input — dense read/scale/write, no collective, no barrier)
      layer 2: x2' = Dis*relu(h1)  (2 chunked AllGathers mid-L1, ~35us each)
    so the two layers share ONE host-side edge plan (same idx/dr arrays,
    same one-hot structure), need no per-edge scale pass (dis[src] lives in
    the table, dis[dst] is applied post-aggregation via a replicated row
    table), and skip the old f32->bf16 ScalarE conversion entirely.
  - One-hots [edge, dst, tile] built in a single is_equal pass (2x DVE mode).
  - Edges sorted by dst, sharded across 8 cores by dst range (6250/core),
    dst blocks of 128, groups of GS=4 blocks; scatter-add via
    PSUM-accumulated TensorE matmuls; self loops via stage @ identity in
    pass 1; gathers in 8-tile single-packet dma_gather calls round-robined
    over 4 SWDGE queues (measured fastest vs all bigger-call variants).
  - Both layers run two passes (one per chunk table) accumulating into a
    bf16 SBUF accumulator; layer-2 chunk-0 AllGather is emitted mid-layer-1
    pass 1, chunk-1 after layer-2 pass 0's gather desc-gen.
Host-side work is index-only prep (sort/degree-histogram/plan) + output concat.
"""

import numpy as np
import ml_dtypes

import concourse.bass as bass
import concourse.bacc as bacc
import concourse.mybir as mybir
import concourse.tile as tile
from concourse.bass_utils import run_bass_kernel_spmd
from concourse.library_config import mlp
from concourse.masks import make_identity

P = 128
N_NODES = 50000
N_EDGES = 800000
IN_CH = 128
HID_CH = 128
OUT_CH = 64
N_CORES = 8
NSH = N_NODES // N_CORES          # 6250 nodes per core
NBLK = (NSH + P - 1) // P         # 49 dst blocks per core (48 full + 106)
NFULL = NSH // P                  # 48
NTAIL = NSH - NFULL * P           # 106
GS = 4                            # dst blocks per group
NG = (NBLK + GS - 1) // GS        # 13 groups (last has 1 block)
GCAP = 8                          # max tiles per dma_gather call
CLEN = [25, 24]                   # src-block chunks (blocks 0:25, 25:49)
CSTART = [0, 25]
AG_SPLIT = 7                      # x2 chunk-0 AllGather after this many
                                  # layer-1 pass-1 groups (covers blocks 0:28)

BF16 = mybir.dt.bfloat16
F32 = mybir.dt.float32

LAST_RESULT = None  # for test harness: BassKernelResults of last run
REPEAT = 1          # measurement aid: unroll the kernel body this many times


def _plan(cnt):
    t = np.ceil(cnt.max(axis=0) / P).astype(np.int64)
    off = np.concatenate([[0], np.cumsum(t)])[:-1]
    return t, off, int(t.sum())


def _host_prep(edge_index):
    """Index-only preprocessing. Returns per-core upload arrays + tile plan."""
    src = edge_index[0].astype(np.int64)
    dst = edge_index[1].astype(np.int64)

    deg = np.bincount(dst, minlength=N_NODES) + 1
    dis = (1.0 / np.sqrt(deg.astype(np.float64))).astype(np.float64)

    core = dst // NSH
    i_dst = dst - core * NSH
    blk = i_dst // P
    grp = blk // GS
    big = blk - grp * GS

    # full-graph dis in block-column layout: discol[p, k*NBLK+b] = dis[node]
    discol = np.zeros((P, N_CORES * NBLK), np.float32)
    nodes = np.arange(N_NODES)
    kk_ = nodes // NSH
    ii_ = nodes - kk_ * NSH
    discol[ii_ % P, kk_ * NBLK + ii_ // P] = dis[nodes]

    # chunked tables; row = (ksrc*128+psrc)*len_c + (bsrc-cstart)
    ksrc = src // NSH
    isrc = src - ksrc * NSH
    bsrc = isrc // P
    psrc = isrc - bsrc * P
    ch = (bsrc >= CLEN[0]).astype(np.int64)
    lenc = np.where(ch == 1, CLEN[1], CLEN[0])
    rows = (ksrc * P + psrc) * lenc + (bsrc - ch * CSTART[1])
    kk = ((core * NG + grp) * 2 + ch) * GS + big
    NSLOT = NG * 2 * GS                                   # 104 per core
    cnt = np.zeros((N_CORES, NSLOT), np.int64)
    np.add.at(cnt, (core, kk - core * NSLOT), 1)
    t, off, T = _plan(cnt)

    drel = (dst - (core * NSH + blk * P)).astype(np.float32)

    # secondary sort by table row: each gather call's descriptors become
    # ascending in address (better DRAM behavior; order within a section is
    # free since scatter-add commutes)
    order = np.lexsort((rows, kk))
    ks = kk[order]
    group_start = np.concatenate([[0], np.cumsum(cnt.reshape(-1))])[:-1]
    pos = np.arange(len(ks)) - group_start[ks]
    slot_in_core = off[ks % NSLOT] * P + pos
    ecore = ks // NSLOT
    EPC = T * P
    idx_rows = np.zeros((N_CORES, EPC), np.int64)
    dr = np.full((N_CORES, EPC), -1.0, np.float32)
    idx_rows[ecore, slot_in_core] = rows[order]
    dr[ecore, slot_in_core] = drel[order]
    idxw = np.zeros((N_CORES, 16, T * 8), np.int16)
    ii = np.arange(EPC)
    for k in range(N_CORES):
        w = np.zeros((16, T * 8), np.int16)
        w[ii % 16, ii // 16] = idx_rows[k]
        idxw[k] = w
    idxw = np.tile(idxw, (1, 8, 1))
    drw = dr.reshape(N_CORES, T, P).transpose(0, 2, 1)

    disw = np.zeros((N_CORES, P, NBLK), np.float32)
    diswr = np.zeros((N_CORES, NBLK * P), np.float32)
    nodes = np.arange(NBLK * P)
    valid = nodes < NSH
    disf = dis.astype(np.float32)
    for k in range(N_CORES):
        v = np.zeros(NBLK * P, np.float32)
        v[valid] = disf[k * NSH + nodes[valid]]
        disw[k] = v.reshape(NBLK, P).T
        diswr[k] = v

    return {
        "t": t, "off": off, "T": T,
        "idxw": idxw.astype(np.int16),
        "dr": drw.astype(ml_dtypes.bfloat16),
        "disw": disw, "diswr": diswr.astype(ml_dtypes.bfloat16),
        "discol": discol,
    }


def _make_in_maps(x, W1, b1, W2, b2, prep):
    in_maps = []
    xf = np.ascontiguousarray(x, dtype=np.float32)
    for k in range(N_CORES):
        in_maps.append({
            "x": xf,
            "xsh": np.ascontiguousarray(xf[k * NSH:(k + 1) * NSH]),
            "discol": np.ascontiguousarray(prep["discol"]),
            "w1": np.ascontiguousarray(W1, dtype=np.float32),
            "b1": np.ascontiguousarray(np.asarray(b1, np.float32).reshape(HID_CH, 1)),
            "b1row": np.ascontiguousarray(
                np.tile(np.asarray(b1, np.float32)[None, :], (P, 1))),
            "b2row": np.ascontiguousarray(
                np.tile(np.asarray(b2, np.float32)[None, :], (P, 1))),
            "w2": np.ascontiguousarray(W2, dtype=np.float32),
            "b2": np.ascontiguousarray(np.asarray(b2, np.float32).reshape(OUT_CH, 1)),
            "disw": np.ascontiguousarray(prep["disw"][k]),
            "idxw": np.ascontiguousarray(prep["idxw"][k]),
            "dr": np.ascontiguousarray(prep["dr"][k]),
            "diswr": np.ascontiguousarray(
                np.tile(prep["diswr"][k][None, :], (P, 1))),
        })
    return in_maps


def _build(prep):
    t, off, T = prep["t"], prep["off"], prep["T"]

    nc = bacc.Bacc("TRN2", target_bir_lowering=False, num_devices=N_CORES,
                   num_swdge_queues=4)

    t_x = nc.dram_tensor("x", [N_NODES, IN_CH], F32, kind="ExternalInput")
    t_xsh = nc.dram_tensor("xsh", [NSH, IN_CH], F32, kind="ExternalInput")
    t_discol = nc.dram_tensor("discol", [P, N_CORES * NBLK], F32,
                              kind="ExternalInput")
    t_w1 = nc.dram_tensor("w1", [IN_CH, HID_CH], F32, kind="ExternalInput")
    t_b1row = nc.dram_tensor("b1row", [P, HID_CH], F32, kind="ExternalInput")
    t_w2 = nc.dram_tensor("w2", [HID_CH, OUT_CH], F32, kind="ExternalInput")
    t_b2row = nc.dram_tensor("b2row", [P, OUT_CH], F32, kind="ExternalInput")
    t_disw = nc.dram_tensor("disw", [P, NBLK], F32, kind="ExternalInput")
    t_idx = nc.dram_tensor("idxw", [P, T * 8], mybir.dt.int16, kind="ExternalInput")
    t_dr = nc.dram_tensor("dr", [P, T], BF16, kind="ExternalInput")
    t_diswr = nc.dram_tensor("diswr", [P, NBLK * P], BF16, kind="ExternalInput")
    t_out = nc.dram_tensor("out", [NSH, OUT_CH], F32, kind="ExternalOutput")

    x1f = [nc.dram_tensor(f"x1f_{c}", [N_CORES * P, CLEN[c] * IN_CH], BF16)
           for c in range(2)]
    x2s = [nc.dram_tensor(f"x2s_{c}", [P, CLEN[c] * HID_CH], BF16) for c in range(2)]
    # Shared addr_space enables the 1-hop shared-output AllGather path
    x2f = [nc.dram_tensor(f"x2f_{c}", [N_CORES * P, CLEN[c] * HID_CH], BF16,
                          addr_space="Shared")
           for c in range(2)]

    rg = [list(range(N_CORES))]

    def slot(g, c, big):
        return (g * 2 + c) * GS + big

    meta = []
    for g in range(NG):
        bigs = list(range(min(GS, NBLK - g * GS)))
        Tg = int(off[slot(g, 0, 0)])
        S = [0, 0]
        rel = {}
        pos = 0
        for c in range(2):
            for big in bigs:
                n = int(t[slot(g, c, big)])
                rel[(c, big)] = (pos, n)
                S[c] += n
                pos += n
        meta.append((Tg, S, rel, bigs))
    SMAX = max(max(S) for _, S, _, _ in meta)

    with tile.TileContext(nc) as tc:
        with (
            tc.tile_pool(name="const", bufs=1) as cp,
            tc.tile_pool(name="stage", bufs=2) as stp,
            tc.tile_pool(name="sbuf", bufs=4) as sb,
            tc.tile_pool(name="gb", bufs=4) as gb,      # gathered bf16 tiles
            tc.tile_pool(name="ob", bufs=3) as ob,      # one-hot blocks
            tc.tile_pool(name="psum", bufs=3, space="PSUM") as ps,
            tc.tile_pool(name="psA", bufs=4, space="PSUM") as psA,
        ):
            nc.gpsimd.load_library(mlp)

            idx_sb = cp.tile([P, T * 8], mybir.dt.int16)
            discol_sb = cp.tile([P, N_CORES * NBLK], F32)
            nc.sync.dma_start(out=discol_sb[:], in_=t_discol[:, :])
            # dr as [P, 1, T] so middle-dim broadcast slices need no None-axis
            dr_sb = cp.tile([P, 1, T], BF16)
            nc.sync.dma_start(out=dr_sb[:, 0, :], in_=t_dr[:, :])
            # dis[dst] in row layout (replicated over partitions): applied
            # post-aggregation per block
            diswr_sb = cp.tile([P, NBLK * P], BF16)
            disw_sb = cp.tile([P, NBLK], F32)
            nc.sync.dma_start(out=disw_sb[:, :], in_=t_disw[:, :])

            iota_i = cp.tile([P, P], mybir.dt.int32)
            nc.gpsimd.iota(iota_i[:], pattern=[[1, P]], base=0, channel_multiplier=0)
            iota_bf = cp.tile([P, 1, P], BF16)
            nc.vector.tensor_copy(out=iota_bf[:, 0, :], in_=iota_i[:])
            # materialized iota over the dst (middle) axis: val[p, d, j] = d
            iota_big = cp.tile([P, P, SMAX], BF16)
            nc.vector.tensor_copy(
                out=iota_big[:],
                in_=iota_bf[:, 0, :][:, :, None].to_broadcast([P, P, SMAX]))

            ident_bf = cp.tile([P, P], BF16)
            make_identity(nc, ident_bf[:])

            w1_f = cp.tile([IN_CH, HID_CH], F32)
            nc.sync.dma_start(out=w1_f[:], in_=t_w1[:, :])
            w1_bf = cp.tile([IN_CH, HID_CH], BF16)
            nc.vector.tensor_copy(out=w1_bf[:], in_=w1_f[:])
            w2_f = cp.tile([HID_CH, OUT_CH], F32)
            nc.sync.dma_start(out=w2_f[:], in_=t_w2[:, :])
            w2_bf = cp.tile([HID_CH, OUT_CH], BF16)
            nc.vector.tensor_copy(out=w2_bf[:], in_=w2_f[:])
            b1row_sb = cp.tile([P, HID_CH], F32)
            nc.sync.dma_start(out=b1row_sb[:], in_=t_b1row[:, :])
            b2row_sb = cp.tile([P, OUT_CH], F32)
            nc.sync.dma_start(out=b2row_sb[:], in_=t_b2row[:, :])

            x1stage = cp.tile([P, NBLK, IN_CH], BF16)
            x2st0 = cp.tile([P, CLEN[0], HID_CH], BF16)
            x2st1 = cp.tile([P, CLEN[1], HID_CH], BF16)
            x2st = [x2st0, x2st1]

            def x2stage_slice(b):
                c = 0 if b < CSTART[1] else 1
                return x2st[c][:, b - CSTART[c], :]

            acc1 = cp.tile([HID_CH, NBLK, P], BF16)
            acc2 = cp.tile([HID_CH, NBLK, P], BF16)
            acc = [acc1, acc2]

            gq = [0]

            def gather(out_ap, in_ap, t0, n):
                o0 = 0
                while o0 < n:
                    m = min(GCAP, n - o0)
                    nc.gpsimd.dma_gather(
                        out_ap=out_ap[:, o0:o0 + m, :], in_ap=in_ap,
                        idxs_ap=idx_sb[:, 8 * (t0 + o0): 8 * (t0 + o0 + m)],
                        num_idxs=m * P, num_idxs_reg=m * P, elem_size=IN_CH,
                        queue_num=gq[0] % 4, single_packet=True,
                    )
                    gq[0] += 1
                    o0 += m

            def onehot(S, Tg):
                """[P(edge), P(dst), S(tile)] one-hot."""
                o3 = ob.tile([P, P, S], BF16, tag="o3")
                nc.vector.tensor_tensor(
                    out=o3[:], in0=iota_big[:, :, :S],
                    in1=dr_sb[:, :, Tg:Tg + S].to_broadcast([P, P, S]),
                    op=mybir.AluOpType.is_equal,
                )
                return o3

            def epi1(b, ups):
                # ups: [P dst, HID] psum. x2' = Dis*relu(t+b1), relu/scale
                # commute (dis>0) so ACT applies the per-partition dis scale.
                tmp2 = sb.tile([P, HID_CH], F32, tag="ep1")
                nc.vector.tensor_tensor(out=tmp2[:], in0=ups[:],
                                        in1=b1row_sb[:],
                                        op=mybir.AluOpType.add)
                nc.scalar.activation(out=x2stage_slice(b), in_=tmp2[:],
                                     func=mybir.ActivationFunctionType.Relu,
                                     scale=disw_sb[:, b: b + 1])

            def epi2(b, ups):
                tmp2 = sb.tile([P, OUT_CH], F32, tag="ep2")
                nc.vector.tensor_tensor(out=tmp2[:], in0=ups[:],
                                        in1=b2row_sb[:],
                                        op=mybir.AluOpType.add)
                outt = sb.tile([P, OUT_CH], F32, tag="outt")
                nc.scalar.activation(out=outt[:], in_=tmp2[:],
                                     func=mybir.ActivationFunctionType.Relu)
                nb = P if b < NFULL else NTAIL
                nc.sync.dma_start(out=t_out[b * P: b * P + nb, :], in_=outt[:nb, :])

            def do_group(l, tabs, stage_slice, wl, epi, g, cpass):
                Tg, S, rel, bigs = meta[g]
                Sc = S[cpass]
                base = Tg + (S[0] if cpass == 1 else 0)
                if Sc:
                    g2 = gb.tile([P, Sc, HID_CH], BF16, tag="gb")
                    gather(g2[:, :, :], tabs[cpass], base, Sc)
                    o3 = onehot(Sc, base)
                for big in bigs:
                    b = g * GS + big
                    p0, n = rel[(cpass, big)]
                    p0 -= S[0] if cpass == 1 else 0
                    if cpass == 0:
                        if n == 0:
                            nc.vector.memset(acc[l][:, b, :], 0.0)
                            continue
                        tps = psA.tile([HID_CH, P], F32, tag="tps")
                        for ji in range(n):
                            nc.tensor.matmul(out=tps[:], lhsT=g2[:, p0 + ji, :],
                                             rhs=o3[:, :, p0 + ji],
                                             start=(ji == 0), stop=(ji == n - 1))
                        nc.vector.tensor_tensor(
                            out=acc[l][:, b, :], in0=tps[:],
                            in1=diswr_sb[:, b * P:(b + 1) * P],
                            op=mybir.AluOpType.mult)
                    else:
                        tps = psA.tile([HID_CH, P], F32, tag="tps")
                        for ji in range(n):
                            nc.tensor.matmul(out=tps[:], lhsT=g2[:, p0 + ji, :],
                                             rhs=o3[:, :, p0 + ji],
                                             start=(ji == 0), stop=False)
                        nc.tensor.matmul(out=tps[:], lhsT=stage_slice(b),
                                         rhs=ident_bf[:], start=(n == 0),
                                         stop=True)
                        tmp = sb.tile([HID_CH, P], F32, tag="tmp")
                        nc.vector.tensor_tensor(
                            out=tmp[:], in0=tps[:],
                            in1=diswr_sb[:, b * P:(b + 1) * P],
                            op=mybir.AluOpType.mult)
                        t_sb = sb.tile([HID_CH, P], BF16, tag="tsb")
                        nc.vector.tensor_tensor(
                            out=t_sb[:], in0=tmp[:], in1=acc[l][:, b, :],
                            op=mybir.AluOpType.add)
                        ups = ps.tile([P, wl[1]], F32, tag="ups")
                        nc.tensor.matmul(out=ups[:], lhsT=t_sb[:], rhs=wl[0][:],
                                         start=True, stop=True)
                        epi(b, ups)

            tab1 = [x1f[c].ap().rearrange("q (r c) -> (q r) c", c=IN_CH)
                    for c in range(2)]
            tab2 = [x2f[c].ap().rearrange("q (r c) -> (q r) c", c=HID_CH)
                    for c in range(2)]
            x1stage_slice = lambda b: x1stage[:, b, :]

            x1fv = [x1f[c].ap().rearrange("q (b c) -> q b c", c=IN_CH)
                    for c in range(2)]
            RUNS = [[(0, 13), (13, 12)],
                    [(25, 12), (37, 12)]]
            QS = [(0, 13), (13, 12), (25, 12), (37, 12)]
            for _rep in range(REPEAT):
                # layer-1 table x1' = Dis*x, built LOCALLY on every core from
                # the replicated x input (no collective): dense chunked
                # read -> scale -> bf16 -> strided write into table layout
                for c in range(2):
                    for k in range(N_CORES):
                        for b0, run in RUNS[c]:
                            base = k * NSH + b0 * P
                            bx = stp.tile([P, 13, IN_CH], F32, tag="bx1")
                            if b0 + run == NBLK:
                                nc.vector.memset(bx[:, run - 1, :], 0.0)
                                nc.sync.dma_start(
                                    out=bx[:, :run - 1, :],
                                    in_=t_x[base: base + (run - 1) * P, :]
                                    .rearrange("(b p) c -> p b c", p=P))
                                nc.sync.dma_start(
                                    out=bx[:NTAIL, run - 1, :],
                                    in_=t_x[base + (run - 1) * P:
                                            (k + 1) * NSH, :])
                            else:
                                nc.sync.dma_start(
                                    out=bx[:, :run, :],
                                    in_=t_x[base: base + run * P, :]
                                    .rearrange("(b p) c -> p b c", p=P))
                            bxh = stp.tile([P, 13, IN_CH], BF16, tag="bx1h")
                            nc.vector.tensor_tensor(
                                out=bxh[:, :run, :], in0=bx[:, :run, :],
                                in1=discol_sb[:, k * NBLK + b0:
                                              k * NBLK + b0 + run][:, :, None]
                                .to_broadcast([P, run, IN_CH]),
                                op=mybir.AluOpType.mult)
                            nc.sync.dma_start(
                                out=x1fv[c][k * P:(k + 1) * P,
                                            b0 - CSTART[c]:
                                            b0 - CSTART[c] + run, :],
                                in_=bxh[:, :run, :])

                # heavy const loads AFTER the x1f build emission: they are
                # not needed until the first gather/epilogue, so keeping them
                # off the HWDGE queues ahead of the build shortens the head
                nc.sync.dma_start(out=idx_sb[:], in_=t_idx[:, :])
                nc.sync.dma_start(out=diswr_sb[:], in_=t_diswr[:, :])

                # own-shard stage (self loops): Dis*x in [node, block, ch]
                for q0, qn in QS:
                    sbx = stp.tile([P, 13, IN_CH], F32, tag="sbx")
                    if q0 + qn == NBLK:
                        nc.vector.memset(sbx[:, qn - 1, :], 0.0)
                        nc.sync.dma_start(
                            out=sbx[:, :qn - 1, :],
                            in_=t_xsh[q0 * P: (q0 + qn - 1) * P, :]
                            .rearrange("(b p) c -> p b c", p=P))
                        nc.sync.dma_start(out=sbx[:NTAIL, qn - 1, :],
                                          in_=t_xsh[(q0 + qn - 1) * P:, :])
                    else:
                        nc.sync.dma_start(
                            out=sbx[:, :qn, :],
                            in_=t_xsh[q0 * P: (q0 + qn) * P, :]
                            .rearrange("(b p) c -> p b c", p=P))
                    nc.vector.tensor_tensor(
                        out=x1stage[:, q0:q0 + qn, :], in0=sbx[:, :qn, :],
                        in1=disw_sb[:, q0:q0 + qn][:, :, None]
                        .to_broadcast([P, qn, IN_CH]),
                        op=mybir.AluOpType.mult)

                # ================= Layer 1 =================
                for g in range(NG):
                    do_group(0, tab1, x1stage_slice, (w1_bf, HID_CH), epi1, g, 0)
                for g in range(AG_SPLIT):
                    do_group(0, tab1, x1stage_slice, (w1_bf, HID_CH), epi1, g, 1)
                # chunk-0 x2' ready (blocks 0:28 done): exchange while the
                # rest of layer-1 pass 1 runs
                nc.sync.dma_start(
                    out=x2s[0][:, :], in_=x2st[0][:].rearrange("p b c -> p (b c)"))
                nc.gpsimd.collective_compute(
                    "AllGather", mybir.AluOpType.bypass, replica_groups=rg,
                    ins=[x2s[0].ap().opt()], outs=[x2f[0].ap().opt()],
                )
                for g in range(AG_SPLIT, NG):
                    do_group(0, tab1, x1stage_slice, (w1_bf, HID_CH), epi1, g, 1)
                nc.sync.dma_start(
                    out=x2s[1][:, :], in_=x2st[1][:].rearrange("p b c -> p (b c)"))

                # ================= Layer 2 =================
                for g in range(NG):
                    do_group(1, tab2, x2stage_slice, (w2_bf, OUT_CH), epi2, g, 0)
                # chunk-1 AllGather after pass-0's gather desc-gen so those
                # gathers aren't queued behind it on the Pool engine
                nc.gpsimd.collective_compute(
                    "AllGather", mybir.AluOpType.bypass, replica_groups=rg,
                    ins=[x2s[1].ap().opt()], outs=[x2f[1].ap().opt()],
                )
                for g in range(NG):
                    do_group(1, tab2, x2stage_slice, (w2_bf, OUT_CH), epi2, g, 1)

    nc.compile()
    return nc


_BUILD_CACHE = {}


def kernel(x, edge_index, W1, b1, W2, b2, _trace=False):
    global LAST_RESULT
    x = np.asarray(x, dtype=np.float32)
    edge_index = np.asarray(edge_index, dtype=np.int32)

    # memoize prep+program per edge structure: repeated harness calls skip
    # host planning and Bass program construction (the NEFF itself is also
    # cached downstream by the compiler cache)
    ekey = hash(edge_index.tobytes())
    if ekey not in _BUILD_CACHE:
        prep = _host_prep(edge_index)
        _BUILD_CACHE[ekey] = (prep, _build(prep))
    prep, nc = _BUILD_CACHE[ekey]
    in_maps = _make_in_maps(x, W1, b1, W2, b2, prep)

    res = None
    for attempt in range(3):
        try:
            res = run_bass_kernel_spmd(nc, in_maps, core_ids=list(range(N_CORES)),
                                       trace=_trace)
            break
        except Exception:
            # transient axon/device hiccups (e.g. NRT_EXEC_UNIT_UNRECOVERABLE
            # left by a prior run) usually clear on re-execution — but the
            # poisoned PJRT client must be dropped first
            if attempt == 2:
                raise
            import time as _time
            try:
                import jax as _jax
                _jax.clear_caches()
                _jax.extend.backend.clear_backends()
            except Exception:
                pass
            _time.sleep(10)
    LAST_RESULT = res
    out = np.concatenate([res.results[k]["out"] for k in range(N_CORES)], axis=0)
    return out.astype(np.float32)
